# revision 8
# baseline (speedup 1.0000x reference)
"""Trainium2 Bass kernel for nn_DivEncoder (grouped MLP + ELU + L2 norm).

Math (per batch row n):
  z = W1 xg + b1 (per group);  y_d = b2_d + sum_u W2[d,u] elu(z[d,u])
  out = y / max(||y||, eps)

Decomposition with q = relu(-z) (= -min(z,0)):
  elu(z) = z + q + e - 1,  e = exp(-q)
  y = c0 + (wlin x) + sum_u W2 q + sum_u W2 e
  c0 = b2 + sum_u W2 b1 - sum_u W2 ;  wlin = sum_u W2[d,u] W1[d,u,:]

exp via two-point Schraudolph bit-trick (int16 bitcast fp16), all on
cheap 4x-mode DVE/Pool tensor_scalar ops instead of ACT:
  e_k = bitcast_f16(int16_sat(rint(-C1 q + C2 + off_k))),  k=0,1
  e   = wfold (e_0 + e_1);  C1 = 1024/ln2, C2 = 15*1024 + SIGMA
  host folds wfold into the W2e weights (rel err ~4.5e-3 vs 2e-2 gate).

Sharding: batch rows across 8 cores (512 rows each); weights replicated.
Host pre-pass: x cast fp16, transposed per core into feature-major chunks
xt[c] = [128 feats, 512 batch]; fp16 weights packed into one DRAM tensor.

Per-core dataflow, 64 chunks of 128 features (8 groups of 16 v's):
  - L1: 4 fp16 matmuls per chunk (K=32 strips, tile_position packed) into
    2 PSUM tiles [128,1024]; 4 zero-padded matmuls add b1 (bias-as-matmul).
  - EXTR: q = relu(-z), split DVE (tensor_scalar min/mult, 1x from PSUM)
    vs ACT (Relu scale=-1) per-chunk to balance engines.
  - EXP: conv1/conv2 (tensor_scalar fp16->int16, 4x ~615ns) + fadd
    (tensor_tensor fp16, 2x) on DVE/Pool; optionally true ACT exp for a
    few chunks; optionally fadd elided via doubled W2e matmuls (FADD=pe).
  - L2: W2e on e-sum, W2q on q (M=32 col-tiled lhsT) accumulate 16 chunks
    into one PSUM bank; wlin matmul (M=128) adds the linear term.
  - Per bank: evac (+c0), 4 PE transposes, merge copies to batch-major;
    tail: square+row-sum (ACT), rsqrt + Newton (DVE), scale, DMA out.

Engine budget measured on HW (long-loop slope; axon wall too noisy for
single runs): see micro.py. DVE 4x confirmed for tensor_scalar fp16;
tensor_tensor only 2x; ACT exp 1.91us/chunk; relu/copy/square share every
ACT table set so no reloads in steady state.
"""
import sys
sys.path.insert(0, "/opt/trn_rl_repo")

import numpy as np
import ml_dtypes

import concourse.bass as bass
import concourse.bacc as bacc
import concourse.mybir as mybir
import concourse.tile as tile
from concourse import bass_utils

F32 = mybir.dt.float32
F16 = mybir.dt.float16
I16 = mybir.dt.int16
AL = mybir.AluOpType
AF = mybir.ActivationFunctionType

N, H, D, U, V = 4096, 8192, 512, 64, 16
NCORE = 8
R = N // NCORE          # 512 batch rows per core
CH = H // 128           # 64 chunks
BG = 4                  # bank groups (16 chunks each)
EPS = 1e-12

NKIND = 4               # w1a, wl1, w2e, w2q
WBLK = 4                # chunks per weight-block DMA
NBLK = CH // WBLK       # 16 block DMAs
BLKC = NKIND * WBLK * 128   # 2048 cols per block

# Schraudolph two-point exp constants (host-validated: rel err 4.5e-3)
SC1 = 1024.0 / np.log(2.0)
SIGMA = -25.0
SC2 = 15.0 * 1024.0 + SIGMA
SOFF = 427
WFOLD = 1.0 / (float(np.array([round(SC2)], dtype=np.int16).view(np.float16)[0])
               + float(np.array([round(SC2) + SOFF],
                                dtype=np.int16).view(np.float16)[0]))

_cache = {}
import os as _os


def _env(k, d):
    return int(_os.environ.get(k, str(d)))


EXTR_A = _env("EXTR_A", 26)        # chunks with ACT extraction
EXP_A = _env("EXP_A", 32)           # chunks with true ACT exp
CONV_P = _env("CONV_P", 0)        # conv ops (of 2*(64-EXP_A)) on Pool
FADD_P = _env("FADD_P", 32)         # fadds on Pool
FADD_PE = _env("FADD_PE", 0)       # 1 = elide fadd, double W2e matmuls
EXP_DEFER = _env("EXP_DEFER", 0)
EM_DEFER = _env("EM_DEFER", 2)
FIN_DEFER_A = _env("FIN_DEFER_A", 4)
FIN_DEFER_B = _env("FIN_DEFER_B", 6)
FIN_ACT = _env("FIN_ACT", 1)       # evac (+c0) on ACT instead of DVE
MERGE_ACT = _env("MERGE_ACT", 1)   # merge copies on ACT instead of DVE
NORM_MAGIC = _env("NORM_MAGIC", 1)  # rsqrt bit-trick on DVE (no ACT sqrt)
XTR_BUFS = _env("XTR_BUFS", 6)
TAIL_DVE = _env("TAIL_DVE", 0)
POP_FIRST = _env("POP_FIRST", 0)
XDMA = _os.environ.get("XDMA", "alt")  # sync|alt|gps
QBUFS = _env("QBUFS", 4)
ABL_EXTR = _env("ABL_EXTR", 0)
ABL_EXP = _env("ABL_EXP", 0)
ABL_EM = _env("ABL_EM", 0)
ABL_L1 = _env("ABL_L1", 0)
ABL_WL = _env("ABL_WL", 0)


def _spread(count, total=CH):
    """Evenly spread `count` picks over range(total) (bresenham)."""
    picks = set()
    if count <= 0:
        return picks
    acc = 0
    for c in range(total):
        acc += count
        if acc >= total:
            acc -= total
            picks.add(c)
    return picks


ACT_EXTR_SET = _spread(EXTR_A)
ACT_EXP_SET = _spread(EXP_A)


def _conv_engine_plan():
    """Assign each conv op (2 per schr chunk) to DVE or Pool, spread."""
    schr = [c for c in range(CH) if c not in ACT_EXP_SET]
    nops = 2 * len(schr)
    pool_ops = _spread(min(CONV_P, nops), nops)
    plan = {}
    k = 0
    for c in schr:
        plan[c] = (k in pool_ops, (k + 1) in pool_ops)
        k += 2
    return plan


CONV_PLAN = _conv_engine_plan()
# pool fadds spread over the schr chunks only (ACT-exp chunks have no fadd)
_SCHR = [c for c in range(CH) if c not in ACT_EXP_SET]
FADD_POOL_SET = {_SCHR[i] for i in _spread(min(FADD_P, len(_SCHR)),
                                           max(len(_SCHR), 1))}


def _wview(t_wall, c, kind):
    base = BLKC * (c // WBLK) + WBLK * 128 * kind + 128 * (c % WBLK)
    return t_wall[:, base:base + 128]


def _build(loop_reps=1, loop_all=False):
    nc = bacc.Bacc("TRN2", target_bir_lowering=False, debug=False,
                   enable_asserts=False, num_devices=NCORE)
    ap = {}
    ap["xt"] = nc.dram_tensor("xt", [CH, 128, R], F16, kind="ExternalInput").ap()
    ap["wall"] = nc.dram_tensor("wall", [NBLK, 128, BLKC], F16,
                                kind="ExternalInput").ap()
    ap["smalls"] = nc.dram_tensor("smalls", [128, 132], F32,
                                  kind="ExternalInput").ap()
    ap["b1r"] = nc.dram_tensor("b1r", [128, CH * 128], F16,
                               kind="ExternalInput").ap()
    y_out = nc.dram_tensor("y", [R, D], F32, kind="ExternalOutput").ap()

    with tile.TileContext(nc) as tc:
        _emit(nc, tc, ap, y_out, loop_reps, loop_all)
    nc.compile()
    return nc


def _emit(nc, tc, ap, y_out, loop_reps=1, loop_all=False):
    with (
        tc.tile_pool(name="wres", bufs=1) as wres,
        tc.tile_pool(name="xtr", bufs=XTR_BUFS) as xtr,
        tc.tile_pool(name="qp", bufs=QBUFS) as qpool,
        tc.tile_pool(name="esp", bufs=QBUFS) as espool,
        tc.tile_pool(name="eip", bufs=(4 if FADD_PE else 2)) as eipool,
        tc.tile_pool(name="yfm", bufs=1) as yfm,
        tc.tile_pool(name="zps", bufs=(3 if EXTR_A in (0, CH) else 2),
                     space="PSUM") as zps,
        tc.tile_pool(name="zps2", bufs=1, space="PSUM") as zps2,
        tc.tile_pool(name="yps", bufs=2, space="PSUM") as yps,
        tc.tile_pool(name="sml", bufs=1) as sml,
    ):
        t_wall = wres.tile([128, NBLK * BLKC], F16, tag="wall", name="wall")
        t_small = wres.tile([128, 132], F32, tag="smalls", name="smalls")
        t_b1r = wres.tile([128, CH * 128], F16, tag="b1r", name="b1r")
        t_ones = wres.tile([128, 512], F16, tag="ones", name="ones")
        nc.gpsimd.memset(t_ones[:], 0.0)
        for k4 in range(4):
            nc.gpsimd.memset(t_ones[32 * k4:32 * k4 + 1, :], 1.0)
        t_stat = None
        if ABL_EXTR or ABL_EXP:
            t_stat = wres.tile([128, 2048], F16, tag="stat", name="stat")
            nc.gpsimd.memset(t_stat[:], 0.5)

        def load_weights():
            nc.gpsimd.dma_start(t_small[:], ap["smalls"][:])
            q = CH * 128 // 4
            for i4 in range(4):
                nc.gpsimd.dma_start(t_b1r[:, q * i4:q * (i4 + 1)],
                                    ap["b1r"][:, q * i4:q * (i4 + 1)])
            for b in range(NBLK):
                nc.gpsimd.dma_start(t_wall[:, BLKC * b:BLKC * (b + 1)],
                                    ap["wall"][b])

        def c0v(b):
            return t_small[:, b:b + 1]

        t_id = t_small[:, 4:132]

        import contextlib
        loop_cm = tc.For_i(0, loop_reps, 1) if loop_reps > 1 else contextlib.nullcontext()
        if not loop_all:
            load_weights()
        with loop_cm:
            if loop_all:
                load_weights()
            y_banks = {}
            t_yfm = [yfm.tile([128, 512], F32, tag=f"yfm{b}", name=f"yfm{b}")
                     for b in range(BG)]
            t_yTb = yfm.tile([128, 2048], F32, tag="yTb", name="yTb")
            t_yT = [t_yTb[:, 512 * j:512 * (j + 1)] for j in range(4)]

            sched = {}

            def at(it, fn):
                sched.setdefault(it, []).append(fn)

            def make_fin_a(b):
                def fin_a():
                    ybk = y_banks[b]
                    if FIN_ACT:
                        nc.scalar.activation(t_yfm[b][:], ybk[:], AF.Identity,
                                             bias=c0v(b)[:, 0:1])
                    else:
                        nc.vector.tensor_scalar(t_yfm[b][:], ybk[:],
                                                c0v(b)[:, 0:1], None, AL.add)
                    pTg = zps.tile([128, 512], F32, tag="z", name=f"pTg{b}")
                    for j in range(4):
                        nc.tensor.transpose(pTg[:, 128 * j:128 * (j + 1)],
                                            t_yfm[b][:, 128 * j:128 * (j + 1)],
                                            t_id[:])
                    y_banks[b] = None
                    return pTg
                holder = {}

                def run_a():
                    holder["pTg"] = fin_a()

                def run_b():
                    pTg = holder["pTg"]
                    dst = t_yTb[:].rearrange(
                        "p (j g) -> p j g", j=4)[:, :, 128 * b:128 * b + 128]
                    src = pTg[:].rearrange("p (j f) -> p j f", j=4)
                    if MERGE_ACT:
                        nc.scalar.copy(dst, src)
                    else:
                        nc.vector.tensor_copy(dst, src)
                return run_a, run_b

            def make_exp(c, q_t):
                def exp_ops():
                    if ABL_EXP:
                        holders[c]["es"] = [t_stat]
                        return
                    if c in ACT_EXP_SET:
                        es = espool.tile([128, 2048], F16, tag="es",
                                         name=f"es{c}")
                        nc.scalar.activation(es[:], q_t[:], AF.Exp, scale=-1.0)
                        es_views = [es]
                    else:
                        p1, p2 = CONV_PLAN[c]
                        e1 = eipool.tile([128, 2048], I16, tag="e1",
                                         name=f"e1_{c}")
                        e2 = eipool.tile([128, 2048], I16, tag="e2",
                                         name=f"e2_{c}")
                        eng1 = nc.gpsimd if p1 else nc.vector
                        eng2 = nc.gpsimd if p2 else nc.vector
                        eng1.tensor_scalar(e1[:], q_t[:], -SC1, SC2,
                                           AL.mult, AL.add)
                        eng2.tensor_scalar(e2[:], q_t[:], -SC1, SC2 + SOFF,
                                           AL.mult, AL.add)
                        if FADD_PE:
                            es_views = [e1.bitcast(F16), e2.bitcast(F16)]
                        else:
                            es = espool.tile([128, 2048], F16, tag="es",
                                             name=f"es{c}")
                            feng = nc.gpsimd if c in FADD_POOL_SET else nc.vector
                            feng.tensor_tensor(es[:], e1[:].bitcast(F16),
                                               e2[:].bitcast(F16), AL.add)
                            es_views = [es]
                    holders[c]["es"] = es_views
                return exp_ops

            holders = {}

            def make_em(c, q_t):
                def em_mms():
                    b = c // 16
                    ybk = y_banks[b]
                    last_chunk = (c % 16 == 15)
                    es_views = holders.pop(c)["es"]
                    if ABL_EM:
                        return
                    for k in range(4):
                        ysl = ybk[32 * k:32 * k + 32, :]
                        qsl = q_t[:, 512 * k:512 * k + 512]
                        nc.tensor.matmul(
                            ysl, _wview(t_wall, c, 3)[:, 32 * k:32 * k + 32],
                            qsl, start=False, stop=False,
                            tile_position=(0, 32 * k), skip_group_check=True)
                        for vi, ev in enumerate(es_views):
                            esl = ev[:, 512 * k:512 * k + 512]
                            nc.tensor.matmul(
                                ysl, _wview(t_wall, c, 2)[:, 32 * k:32 * k + 32],
                                esl, start=False,
                                stop=(last_chunk and k == 3
                                      and vi == len(es_views) - 1),
                                tile_position=(0, 32 * k), skip_group_check=True)
                return em_mms

            for c in range(CH + 8):
                if POP_FIRST == 1:
                    for fn in sched.pop(c, []):
                        fn()
                xfT = None
                if c < CH:
                    b = c // 16
                    cp = c % 16
                    if cp == 0:
                        y_banks[b] = yps.tile([128, 512], F32, tag="ybank",
                                              name=f"ybank{b}")
                    ybank = y_banks[b]
                    holders[c] = {}

                    # --- load x chunk: feature-major [128, 512] fp16
                    xfT = xtr.tile([128, 512], F16, tag="xfT", name=f"xfT{c}")
                    if XDMA == "sync":
                        leng = nc.sync
                    elif XDMA == "gps":
                        leng = nc.gpsimd if (c % 2 == 0) else nc.sync
                    else:
                        leng = nc.scalar if (c % 2 == 0) else nc.sync
                    leng.dma_start(xfT[:], ap["xt"][c])
                if POP_FIRST == 2:
                    # deferred work (em matmuls of older chunks, exps) ahead
                    # of this chunk's L1 so PE/DVE/ACT never head-of-line
                    # stall on the PSUM ring while ready work exists.
                    for fn in sched.pop(c, []):
                        fn()
                if c < CH:
                    # --- L1: matmuls + bias-as-matmul into 2 PSUM tiles
                    maj_act = EXTR_A >= CH // 2
                    in_maj = (c in ACT_EXTR_SET) == maj_act
                    zpool = zps if (in_maj or EXTR_A in (0, CH)) else zps2
                    zAB = [zpool.tile([128, 1024], F32, tag="z", name=f"z{c}_{h}")
                           for h in range(2)]
                    for k in (range(4) if not ABL_L1 else []):
                        zsl = zAB[k // 2][:, 512 * (k % 2):512 * (k % 2) + 512]
                        row = slice(32 * k, 32 * k + 32)
                        nc.tensor.matmul(zsl, _wview(t_wall, c, 0)[row, :],
                                         xfT[row, :],
                                         start=True, stop=False,
                                         tile_position=(32 * k, 0),
                                         skip_group_check=True)
                        nc.tensor.matmul(
                            zsl,
                            t_b1r[32 * k:32 * k + 32, 128 * c:128 * (c + 1)],
                            t_ones[32 * k:32 * k + 32, :],
                            start=False, stop=True,
                            tile_position=(32 * k, 0),
                            skip_group_check=True)
                    # --- wlin matmul (M=128, zero-padded lhsT, fp16)
                    if not ABL_WL:
                        nc.tensor.matmul(ybank[:, :], _wview(t_wall, c, 1)[:, :],
                                         xfT[:, :],
                                         start=(cp == 0), stop=False,
                                         skip_group_check=True)
                    # --- EXTR: q = relu(-z) (DVE or ACT per plan)
                    q_t = qpool.tile([128, 2048], F16, tag="q", name=f"q{c}")
                    if ABL_EXTR:
                        q_t = t_stat
                    else:
                        for h in range(2):
                            qsl = q_t[:, 1024 * h:1024 * h + 1024]
                            if c in ACT_EXTR_SET:
                                nc.scalar.activation(qsl, zAB[h][:], AF.Relu,
                                                     scale=-1.0)
                            else:
                                nc.vector.tensor_scalar(qsl, zAB[h][:], 0.0,
                                                        -1.0, AL.min, AL.mult)
                    if EXP_DEFER == 0:
                        make_exp(c, q_t)()
                    else:
                        at(c + EXP_DEFER, make_exp(c, q_t))
                    at(max(c + EM_DEFER, c + EXP_DEFER), make_em(c, q_t))
                    if cp == 15:
                        run_a, run_b = make_fin_a(b)
                        at(c + FIN_DEFER_A, run_a)
                        at(c + FIN_DEFER_B, run_b)
                if not POP_FIRST:
                    for fn in sched.pop(c, []):
                        fn()

            # ---- norm + output (batch-major tiles already in t_yT)
            for j in range(4):
                yT = t_yT[j]
                sq = xtr.tile([128, 512], F32, tag="sq", name=f"sq{j}")
                ss = sml.tile([128, 1], F32, tag=f"ss{j}")
                if TAIL_DVE:
                    nc.vector.tensor_tensor(sq[:], yT[:], yT[:], AL.mult)
                    nc.vector.reduce_sum(ss[:], sq[:], axis=mybir.AxisListType.X)
                else:
                    nc.scalar.activation(sq[:], yT[:], AF.Square, accum_out=ss[:])
                if NORM_MAGIC:
                    r0 = sml.tile([128, 1], F32, tag=f"r0{j}")
                    sh = sml.tile([128, 1], mybir.dt.int32, tag=f"sh{j}")
                    nc.vector.tensor_scalar(sh[:], ss[:].bitcast(mybir.dt.int32),
                                            1, None, AL.logical_shift_right)
                    nc.vector.tensor_scalar(sh[:], sh[:], 0, None,
                                            AL.bitwise_not)
                    nc.vector.tensor_scalar(r0[:].bitcast(mybir.dt.int32),
                                            sh[:], 0x5f3759df + 1, None,
                                            AL.add)
                    r1 = sml.tile([128, 1], F32, tag=f"r1{j}")
                    t1 = sml.tile([128, 1], F32, tag=f"t1{j}")
                    cur = r0
                    for it in range(3):
                        nc.vector.tensor_tensor(t1[:], cur[:], cur[:], AL.mult)
                        nc.vector.tensor_tensor(t1[:], t1[:], ss[:], AL.mult)
                        nc.vector.tensor_scalar(t1[:], t1[:], -0.5, 1.5,
                                                AL.mult, AL.add)
                        nxt = r1 if cur is r0 else r0
                        nc.vector.tensor_tensor(nxt[:], cur[:], t1[:], AL.mult)
                        cur = nxt
                    r1 = cur
                else:
                    s = sml.tile([128, 1], F32, tag=f"s{j}")
                    nc.scalar.activation(s[:], ss[:], AF.Sqrt)
                    nc.vector.tensor_scalar(s[:], s[:], float(EPS), None, AL.max)
                    r0 = sml.tile([128, 1], F32, tag=f"r0{j}")
                    nc.vector.reciprocal(r0[:], s[:])
                    t1 = sml.tile([128, 1], F32, tag=f"t1{j}")
                    nc.vector.tensor_tensor(t1[:], r0[:], r0[:], AL.mult)
                    nc.vector.tensor_tensor(t1[:], t1[:], ss[:], AL.mult)
                    nc.vector.tensor_scalar(t1[:], t1[:], -0.5, 1.5, AL.mult, AL.add)
                    r1 = sml.tile([128, 1], F32, tag=f"r1{j}")
                    nc.vector.tensor_tensor(r1[:], r0[:], t1[:], AL.mult)
                if TAIL_DVE:
                    nc.vector.tensor_scalar(yT[:], yT[:], r1[:], None, AL.mult)
                else:
                    nc.scalar.activation(yT[:], yT[:], AF.Copy, scale=r1[:])
                nc.sync.dma_start(y_out[128 * j:128 * (j + 1), :], yT[:])


def _pack_host(W1, b1, W2, b2):
    W1 = W1.astype(np.float32)
    b1 = b1.astype(np.float32)
    W2 = W2.astype(np.float32)
    b2 = b2.astype(np.float32)

    wlin = np.einsum('du,duv->dv', W2.astype(np.float64),
                     W1.astype(np.float64)).astype(np.float32)
    c0 = b2 + (W2 * b1).sum(-1) - W2.sum(-1)

    W1h = W1.astype(np.float16)
    wlh = wlin.astype(np.float16)

    w1hi = np.zeros((CH, 128, 128), np.float16)
    wlhi = np.zeros((CH, 128, 128), np.float16)
    w2e = np.zeros((CH, 128, 128), np.float16)
    w2q = np.zeros((CH, 128, 128), np.float16)
    c0s = np.zeros((BG, 128, 1), np.float32)
    b1c = np.zeros((CH, 128, 4), np.float32)

    for c in range(CH):
        cp = c % 16
        bi = c // 16
        # e-weights: folded for schr chunks, plain for ACT-exp chunks
        ew = (W2 * WFOLD if c not in ACT_EXP_SET else W2).astype(np.float16)
        qw = W2.astype(np.float16)
        for k in range(4):
            g0 = 8 * c + 2 * k
            g1 = g0 + 1
            w1hi[c, 32 * k:32 * k + 16, 0:64] = W1h[g0].T
            w1hi[c, 32 * k + 16:32 * k + 32, 64:128] = W1h[g1].T
            scol = 32 * k + 2 * cp
            wlhi[c, 32 * k:32 * k + 16, scol] = wlh[g0]
            wlhi[c, 32 * k + 16:32 * k + 32, scol + 1] = wlh[g1]
            w2e[c, 0:64, scol] = ew[g0]
            w2e[c, 64:128, scol + 1] = ew[g1]
            w2q[c, 0:64, scol] = qw[g0]
            w2q[c, 64:128, scol + 1] = qw[g1]
            b1c[c, 0:64, k] = b1[g0]
            b1c[c, 64:128, k] = b1[g1]
            c0s[bi, scol, 0] = c0[g0]
            c0s[bi, scol + 1, 0] = c0[g1]
    # bias-as-matmul row table
    b1r = np.zeros((128, CH * 128), np.float16)
    for c in range(CH):
        for k in range(4):
            b1r[32 * k, 128 * c:128 * c + 128] = b1c[c, :, k]

    # permutation matrix: transpose output col j (= d-local) <- slot s
    ident = np.zeros((128, 128), dtype=np.float32)
    for cp in range(16):
        for k in range(4):
            for i_ in range(2):
                jcol = 8 * cp + 2 * k + i_
                slot = 32 * k + 2 * cp + i_
                ident[slot, jcol] = 1.0

    kinds = [w1hi, wlhi, w2e, w2q]
    wall = np.zeros((NBLK, 128, BLKC), np.float16)
    for c in range(CH):
        bi, ci = c // WBLK, c % WBLK
        for k, kt in enumerate(kinds):
            off = WBLK * 128 * k + 128 * ci
            wall[bi, :, off:off + 128] = kt[c]

    # smalls: c0 cols 0..3, ident cols 4..131
    smalls = np.zeros((128, 132), np.float32)
    for b in range(BG):
        smalls[:, b] = c0s[b, :, 0]
    smalls[:, 4:132] = ident
    return {"wall": wall, "smalls": smalls, "b1r": b1r}


def _pack_x(x):
    """Per-core host pre-pass: cast fp16 + transpose to [CH, 128, R]."""
    xt = np.ascontiguousarray(x.T.astype(np.float16)).reshape(CH, 128, R)
    return xt


def kernel(x, W1, b1, W2, b2):
    x = np.asarray(x, dtype=np.float32)
    packed = _pack_host(np.asarray(W1), np.asarray(b1),
                        np.asarray(W2), np.asarray(b2))
    if "nc" not in _cache:
        _cache["nc"] = _build()
    nc = _cache["nc"]
    in_maps = []
    for i in range(NCORE):
        m = dict(packed)
        m["xt"] = _pack_x(x[i * R:(i + 1) * R])
        in_maps.append(m)
    res = bass_utils.run_bass_kernel_spmd(nc, in_maps, core_ids=list(range(NCORE)))
    out = np.concatenate([res.results[i]["y"] for i in range(NCORE)], axis=0)
    return out.astype(np.float32)
